# revision 28
# baseline (speedup 1.0000x reference)
"""Expert-parallel sparse MoE kernel v3 for Trainium2 (8 NeuronCores).

Top-2-of-8 MoE MLP, T=4096 tokens, H=I=1024. Expert-parallel: core c owns
expert c's weights. Host: routing decision (top-2 indices + softmax
weights), token gather/pad, input transpose/layout, output combine
(weighted scatter-add). Device: gate/up projection, gpt_oss GLU, down
projection — all bf16 matmuls with f32 PSUM accumulation.

v4 vs v2 (measured: contend≈nodma => DMA overlaps compute for free in
HW; v2's full-nodma≈37us gap was pure dependency serialization):
- Software-pipelined For_i body: loads hoisted to a prologue before the
  loop; inside the body each weight tile is RE-loaded right after its
  last reader (per-m wgu tiles, per-hb wd tiles, xg after L1) so the
  next iteration's DMA overlaps this iteration's compute. Same DMA +
  compute work per iteration; slope unchanged in meaning.
- For_i(staggered_reset=True): rolling 4-stage sem resets instead of a
  per-iteration all-engine barrier (plain For_i cost ~5us/iter more;
  manual stage_boundary placement was worse than the auto equal split).
- Per-m / per-hb weight tiles (separate tiles => exact WAR tracking).
- Fused GLU epilogue via the min-commute identity
  min(g,L)*sigmoid(a*min(g,L)) == min(g*sigmoid(a*g), L*sigmoid(a*L)):
  ACT Silu(scale=a) reads PSUM directly; DVE does clip+scale, u-clip,
  and (u+1)*glu via scalar_tensor_tensor. 1 ACT + 3 DVE ops per chunk.
- L2 PSUM->SBUF copies on VectorE (idle in L2), y DMA per-hb on the
  ScalarE HWDGE ring (idle in L2).
- PSUM managed as 8 explicit bank tiles, static rotation.

Measured (interleaved same-process A/B, For_i slope): compute-only
(nodma) 87.6-88.7us ~= bf16 PE roofline 87.1us (208,896 streamed
columns at 2.4GHz, 1 col/cycle); full kernel 91.6us in a healthy
process. Device throttles under sustained benching (same NEFF measures
92-120us depending on process/thermal state).
"""

import os
import time

import numpy as np
import ml_dtypes

import concourse.tile as tile
from concourse import bacc, mybir
from concourse.bass_utils import run_bass_kernel_spmd

B, S, H, E, I, K = 2, 2048, 1024, 8, 1024, 2
ALPHA, LIMIT = 1.702, 7.0
# min(g,L)*sigmoid(a*g clipped) == min(silu_a(g)/a, L*sigmoid(a*L))
CLIP_G = LIMIT / (1.0 + np.exp(-ALPHA * LIMIT))
SCLIP = float(ALPHA * CLIP_G)  # clip threshold for silu(a*g)
T = B * S
P = 128
HB = H // P  # 8
IB = I // P  # 8
N_CORES = 8

BF16 = mybir.dt.bfloat16
F32 = mybir.dt.float32
NP_BF16 = ml_dtypes.bfloat16
AF = mybir.ActivationFunctionType
ALU = mybir.AluOpType


def _ceil_to(x, m):
    return ((x + m - 1) // m) * m


def _chunks(total, step):
    out = []
    o = 0
    while o < total:
        w = min(step, total - o)
        out.append((o, w))
        o += w
    return out


def build_expert_kernel(C: int, has_bias: bool, reps: int = 1,
                        bench: bool = False, variant: str = "full"):
    """Per-core Bass graph. C = token capacity (multiple of 16).

    reps>1 wraps the body in a hardware For_i loop (bench only);
    bench=True uses internal DRAM for I/O so host transfer is excluded.
    variant (bench only): phase-ablation for bottleneck isolation —
    full | dmaonly | nodma | l1pe | l1full | l2only | contend."""
    assert C % 16 == 0
    do_dma = variant in ("full", "dmaonly", "l1pe", "l1full")
    do_l1 = variant in ("full", "nodma", "l1pe", "l1full", "contend",
                        "l1ndma")
    do_epi = variant in ("full", "nodma", "l1full", "contend", "l1ndma")
    do_l2 = variant in ("full", "nodma", "l2only", "contend")
    l2_dma = variant == "l2only"
    shadow_dma = variant == "contend"
    pipelined = reps > 1

    nc = bacc.Bacc("TRN2", target_bir_lowering=False, debug=False,
                   num_devices=N_CORES)

    ikind = dict(kind="Internal") if bench else dict(kind="ExternalInput")
    # xg: tokens pre-transposed [P, HB, C]
    xg_ap = nc.dram_tensor("xg", [P, HB * C], BF16, **ikind).ap()
    # wgu: gate+up packed per m-block: [P, IB, 2, HB, 128]
    wgu_ap = nc.dram_tensor("wgu", [P, IB * 2 * HB * P], BF16, **ikind).ap()
    # wd: [P, HB, IB, 128] — per-hb contiguous stationary blocks:
    # wd4[p, hb, ib, h'] = wd[ib*128+p, hb*128+h']
    wd_ap = nc.dram_tensor("wd", [P, HB * IB * P], BF16, **ikind).ap()
    if has_bias:
        bg_ap = nc.dram_tensor("bg", [P, IB], F32, **ikind).ap()
        bu_ap = nc.dram_tensor("bu", [P, IB], F32, **ikind).ap()
    if bench:
        y_ap = nc.dram_tensor("y_int", [P, HB * C], BF16).ap()
        yext_ap = nc.dram_tensor("y", [P, 512], F32, kind="ExternalOutput").ap()
    else:
        y_ap = nc.dram_tensor("y", [P, HB * C], BF16,
                              kind="ExternalOutput").ap()

    xg_r = xg_ap.rearrange("p (hb c) -> p hb c", hb=HB)
    wgu_r = wgu_ap.rearrange("p (m g hb i) -> p m g hb i", m=IB, g=2, hb=HB)
    wd_r = wd_ap.rearrange("p (hb m i) -> p hb m i", hb=HB, m=IB)
    y_r = y_ap.rearrange("p (hb c) -> p hb c", hb=HB)

    nchunks = _chunks(C, 512)
    NCH = len(nchunks)

    with tile.TileContext(nc) as tc:
        with (
            tc.tile_pool(name="weights", bufs=1) as wpool,
            tc.tile_pool(name="xgt", bufs=1) as xpool,
            tc.tile_pool(name="act", bufs=1) as apool,
            tc.tile_pool(name="elem", bufs=3) as epool,
            tc.tile_pool(name="yout", bufs=1) as ypool,
            tc.tile_pool(name="psum", bufs=1, space="PSUM") as pspool,
        ):
            # persistent SBUF tiles — per-m / per-hb weight tiles so WAR
            # deps for the pipelined reloads are exact
            # separate g / u tiles per m: the g-half reload can start as
            # soon as the g-pass finishes (half an m-block earlier than a
            # combined tile), with exact per-tile WAR tracking
            wg_t = [wpool.tile([P, HB, P], BF16, name=f"wg{m}")
                    for m in range(IB)]
            wu_t = [wpool.tile([P, HB, P], BF16, name=f"wu{m}")
                    for m in range(IB)]
            wd_t = [wpool.tile([P, IB, P], BF16, name=f"wd{hb}")
                    for hb in range(HB)]
            xgT = xpool.tile([P, HB, C], BF16)
            actT = apool.tile([P, IB, C], BF16)
            yT = ypool.tile([P, HB, C], BF16)
            ps = [pspool.tile([P, 512], F32, space="PSUM", name=f"ps{b}")
                  for b in range(8)]
            if has_bias:
                bg_sb = wpool.tile([P, IB], F32)
                bu_sb = wpool.tile([P, IB], F32)

            if variant in ("nodma", "l2only", "contend", "l1ndma"):
                nc.vector.memset(xgT[:], 0.01)
                for m in range(IB):
                    nc.vector.memset(wg_t[m][:], 0.01)
                    nc.vector.memset(wu_t[m][:], 0.01)
                for hb in range(HB):
                    nc.vector.memset(wd_t[hb][:], 0.01)
                nc.vector.memset(actT[:], 0.01)
            if shadow_dma:
                sh_xg = wpool.tile([P, HB, C], BF16)
                sh_wg = [wpool.tile([P, HB, P], BF16, name=f"shg{m}")
                         for m in range(IB)]
                sh_wu = [wpool.tile([P, HB, P], BF16, name=f"shu{m}")
                         for m in range(IB)]
                sh_wd = [wpool.tile([P, IB, P], BF16, name=f"shd{hb}")
                         for hb in range(HB)]

            def load_all(xg_dst, wg_dst, wu_dst, wd_dst):
                nc.sync.dma_start(xg_dst[:], xg_r[:])
                for m in range(IB):
                    nc.gpsimd.dma_start(wg_dst[m][:], wgu_r[:, m, 0])
                    nc.gpsimd.dma_start(wu_dst[m][:], wgu_r[:, m, 1])
                for hb in range(HB):
                    nc.scalar.dma_start(wd_dst[hb][:], wd_r[:, hb])

            # prologue loads (outside the loop): iteration 0's data
            if do_dma or l2_dma:
                load_all(xgT, wg_t, wu_t, wd_t)
                if has_bias and do_dma:
                    nc.sync.dma_start(bg_sb[:], bg_ap[:, :])
                    nc.sync.dma_start(bu_sb[:], bu_ap[:, :])

            bank_ctr = [0]

            def next_banks(n):
                out = [ps[(bank_ctr[0] + k) % 8] for k in range(n)]
                bank_ctr[0] += n
                return out

            # explicit stage boundaries need all three call sites to fire,
            # i.e. both the L1 and L2 loops must be present in the variant
            use_sb = (pipelined and do_l1 and do_l2
                      and os.environ.get("BENCH_SB", "0") == "1")

            def body():
                if shadow_dma:
                    load_all(sh_xg, sh_wg, sh_wu, sh_wd)

                # ---- layer 1: actT[i-part, C] ----
                for m in range(IB if do_l1 else 0):
                    if m == IB // 2 and use_sb:
                        # stage 0 | 1 boundary: mid-L1
                        tc.stage_boundary()
                    g_pss = [b[:, :nw] for b, (n0, nw)
                             in zip(next_banks(NCH), nchunks)]
                    u_pss = [b[:, :nw] for b, (n0, nw)
                             in zip(next_banks(NCH), nchunks)]
                    for hb in range(HB):
                        for j, (n0, nw) in enumerate(nchunks):
                            nc.tensor.matmul(
                                g_pss[j],
                                lhsT=wg_t[m][:, hb, :],
                                rhs=xgT[:, hb, n0:n0 + nw],
                                start=(hb == 0), stop=(hb == HB - 1))
                    if do_dma and pipelined:
                        # g-half reload: all its readers (this m's g-pass)
                        # are done; starts half an m-block before the
                        # combined-tile reload could
                        nc.gpsimd.dma_start(wg_t[m][:], wgu_r[:, m, 0])
                    for hb in range(HB):
                        for j, (n0, nw) in enumerate(nchunks):
                            nc.tensor.matmul(
                                u_pss[j],
                                lhsT=wu_t[m][:, hb, :],
                                rhs=xgT[:, hb, n0:n0 + nw],
                                start=(hb == 0), stop=(hb == HB - 1))
                    for j, (n0, nw) in enumerate(nchunks if do_epi else []):
                        g_ps, u_ps = g_pss[j], u_pss[j]
                        if has_bias:
                            nc.vector.tensor_add(
                                g_ps, g_ps,
                                bg_sb[:, m:m + 1].to_broadcast([P, nw]))
                            nc.vector.tensor_add(
                                u_ps, u_ps,
                                bu_sb[:, m:m + 1].to_broadcast([P, nw]))
                        sg = epool.tile([P, 512], BF16, tag="sg",
                                        name="sg")[:, :nw]
                        sgc = epool.tile([P, 512], BF16, tag="sgc",
                                         name="sgc")[:, :nw]
                        uc = epool.tile([P, 512], BF16, tag="uc",
                                        name="uc")[:, :nw]
                        # sg = silu(a*g) = a * g*sigmoid(a*g)
                        nc.scalar.activation(sg, g_ps, AF.Silu, scale=ALPHA)
                        # sgc = min(sg, a*CLIP_G) / a  (= clipped glu)
                        nc.vector.tensor_scalar(
                            sgc, sg, SCLIP, 1.0 / ALPHA, ALU.min, ALU.mult)
                        # uc = clip(u, +-LIMIT)
                        nc.vector.tensor_scalar(
                            uc, u_ps, LIMIT, -LIMIT, ALU.min, ALU.max)
                        # actT = (uc + 1) * sgc
                        nc.vector.scalar_tensor_tensor(
                            actT[:, m, n0:n0 + nw], uc, 1.0, sgc,
                            ALU.add, ALU.mult)
                    if do_dma and pipelined:
                        # u-half reload for the next iteration; overlaps
                        # the rest of L1 + all of L2
                        nc.gpsimd.dma_start(wu_t[m][:], wgu_r[:, m, 1])
                fine = os.environ.get("BENCH_FINE", "1") == "1"
                if do_dma and pipelined:
                    if not do_l1:  # dmaonly ablation: loop above skipped
                        for m in range(IB):
                            nc.gpsimd.dma_start(wg_t[m][:], wgu_r[:, m, 0])
                            nc.gpsimd.dma_start(wu_t[m][:], wgu_r[:, m, 1])
                    if fine:
                        # per-hb reloads: if slice-level WAR tracking is
                        # precise, early hb slices start before L1 fully
                        # ends; never worse than the single DMA
                        for hb in range(HB):
                            nc.sync.dma_start(xgT[:, hb, :], xg_r[:, hb, :])
                    else:
                        nc.sync.dma_start(xgT[:], xg_r[:])

                if use_sb:
                    # stage 1 | 2 boundary: L1 done, xg reload issued —
                    # it gets all of stage 2 (half of L2) to complete
                    tc.stage_boundary()

                # ---- layer 2: yT[h-part, C] ----
                for hb in range(HB if do_l2 else 0):
                    if hb == HB // 2 and use_sb:
                        # stage 2 | 3 boundary: mid-L2
                        tc.stage_boundary()
                    y_pss = [b[:, :nw] for b, (n0, nw)
                             in zip(next_banks(NCH), nchunks)]
                    for ib in range(IB):
                        for j, (n0, nw) in enumerate(nchunks):
                            nc.tensor.matmul(
                                y_pss[j],
                                lhsT=wd_t[hb][:, ib, :],
                                rhs=actT[:, ib, n0:n0 + nw],
                                start=(ib == 0), stop=(ib == IB - 1))
                    for j, (n0, nw) in enumerate(nchunks):
                        nc.vector.tensor_copy(yT[:, hb, n0:n0 + nw],
                                              y_pss[j])
                        if fine and (do_dma or shadow_dma):
                            nc.scalar.dma_start(y_r[:, hb, n0:n0 + nw],
                                                yT[:, hb, n0:n0 + nw])
                    if not fine and (do_dma or shadow_dma):
                        nc.scalar.dma_start(y_r[:, hb, :], yT[:, hb, :])
                    if (do_dma or l2_dma) and pipelined:
                        nc.scalar.dma_start(wd_t[hb][:], wd_r[:, hb])
                if (do_dma or l2_dma) and pipelined and not do_l2:
                    for hb in range(HB):  # l1pe/l1full/dmaonly ablations
                        nc.scalar.dma_start(wd_t[hb][:], wd_r[:, hb])

            if reps == 1:
                body()
            else:
                # staggered_reset: rolling per-stage sem resets instead of
                # a full all-engine barrier at each iteration boundary, so
                # iteration i+1's L1 overlaps iteration i's L2 tail/DMA.
                # BENCH_STAG=0 falls back to plain For_i with BENCH_U
                # bodies per hardware iteration (barrier amortization);
                # total body count stays == reps either way.
                stag = os.environ.get("BENCH_STAG", "1") == "1"
                unroll = int(os.environ.get("BENCH_U", "1"))
                if stag:
                    # unroll>1: emit several bodies per hardware iteration
                    # (leftover bodies run before the loop so the total
                    # body count stays exactly == reps) — halves the
                    # per-body stage-reset overhead and widens the DMA
                    # drain windows
                    main, rem = divmod(reps, unroll)
                    for _ in range(rem):
                        body()
                    with tc.For_i(0, main, staggered_reset=True):
                        for _ in range(unroll):
                            body()
                else:
                    main, rem = divmod(reps, unroll)
                    for _ in range(rem):
                        body()
                    with tc.For_i(0, main):
                        for _ in range(unroll):
                            body()
            if bench:
                ylast = ypool.tile([P, 512], BF16, tag="ylast", name="ylast")
                # scalar queue: FIFO-ordered behind the per-hb y writes
                nc.scalar.dma_start(ylast[:], y_r[:, 0, 0:512])
                yext = ypool.tile([P, 512], F32, tag="yext", name="yext")
                nc.scalar.activation(yext[:], ylast[:], AF.Copy)
                nc.sync.dma_start(yext_ap[:, :], yext[:])

    nc.compile()
    return nc


_KERNEL_CACHE: dict = {}


def build_expert_kernel_replicated(C: int, has_bias: bool, reps: int):
    return build_expert_kernel(C, has_bias, reps, bench=True)


def _get_kernel(C: int, has_bias: bool):
    key = (C, has_bias)
    if key not in _KERNEL_CACHE:
        _KERNEL_CACHE[key] = build_expert_kernel(C, has_bias)
    return _KERNEL_CACHE[key]


def _route(x, router_weight):
    """Host-side top-2 routing + softmax weights (f32, matches
    jax.lax.top_k tie-breaking: stable argsort of -logits)."""
    logits = x @ router_weight
    top2 = np.argsort(-logits, axis=1, kind="stable")[:, :K]
    tv = np.take_along_axis(logits, top2, axis=1)
    mx = tv.max(axis=1, keepdims=True)
    ex = np.exp(tv - mx)
    sm = ex / ex.sum(axis=1, keepdims=True)
    return top2, sm


def prepare_in_maps(hidden_states, router_weight, gate_up_proj,
                    gate_up_proj_bias, down_proj, down_proj_bias):
    x = np.ascontiguousarray(
        np.asarray(hidden_states, dtype=np.float32).reshape(T, H))
    rw = np.asarray(router_weight, dtype=np.float32)
    top2, sm = _route(x, rw)

    idx_lists = []
    wt_lists = []
    for c in range(N_CORES):
        mask = top2 == c
        sel = np.nonzero(mask.any(axis=1))[0]
        idx_lists.append(sel.astype(np.int64))
        wt_lists.append((sm * mask)[sel].sum(axis=1).astype(np.float32))
    max_load = max(len(s) for s in idx_lists)
    C = max(_ceil_to(max_load, 16), 512)

    xbf = x.astype(NP_BF16)
    gup = np.asarray(gate_up_proj, dtype=np.float32)
    gub = np.asarray(gate_up_proj_bias, dtype=np.float32)
    dwn = np.asarray(down_proj, dtype=np.float32)
    dwb = np.asarray(down_proj_bias, dtype=np.float32)
    has_bias = bool(np.any(gub) or np.any(dwb))

    in_maps = []
    for c in range(N_CORES):
        idx = idx_lists[c]
        xg = np.zeros((C, H), dtype=NP_BF16)
        xg[:len(idx)] = xbf[idx]
        # transpose to [P, HB, C]: xgT[p, hb, t] = xg[t, hb*128+p]
        xgT = np.ascontiguousarray(
            xg.reshape(C, HB, P).transpose(2, 1, 0)).reshape(P, HB * C)
        # weights: w2[p, m, hb, i'] = w[hb*128+p, m*128+i']
        wg = np.ascontiguousarray(gup[c, :, 0::2])
        wu = np.ascontiguousarray(gup[c, :, 1::2])
        wd = dwn[c]

        def lay1(w):  # [H, I] -> [P, IB, HB, 128]
            return w.reshape(HB, P, IB, P).transpose(1, 2, 0, 3)

        # wgu packed: [P, IB, 2, HB, 128]
        wgu = np.ascontiguousarray(
            np.stack([lay1(wg), lay1(wu)], axis=2)
        ).astype(NP_BF16).reshape(P, IB * 2 * HB * P)
        # wd4[p, hb, ib, h'] = wd[ib*128+p, hb*128+h']  (per-hb contiguous)
        wd4 = np.ascontiguousarray(
            wd.reshape(IB, P, HB, P).transpose(1, 2, 0, 3)
        ).astype(NP_BF16).reshape(P, HB * IB * P)
        m = {
            "xg": xgT.astype(NP_BF16),
            "wgu": wgu,
            "wd": wd4,
        }
        if has_bias:
            m["bg"] = np.ascontiguousarray(
                gub[c, 0::2].reshape(IB, P).T).astype(np.float32)
            m["bu"] = np.ascontiguousarray(
                gub[c, 1::2].reshape(IB, P).T).astype(np.float32)
        in_maps.append(m)
    return in_maps, idx_lists, wt_lists, C, has_bias


def combine(results, idx_lists, wt_lists, C, down_proj_bias, top2_w=None):
    dwb = np.asarray(down_proj_bias, dtype=np.float32)
    out = np.zeros((T, H), np.float32)
    for c in range(N_CORES):
        idx = idx_lists[c]
        wt = wt_lists[c]
        yT = results[c]["y"].reshape(P, HB, C)
        # yT[p, hb, t] = y[t, hb*128+p]
        y = np.ascontiguousarray(
            yT.transpose(2, 1, 0)).reshape(C, H)[:len(idx)].astype(np.float32)
        if np.any(dwb):
            y = y + dwb[c]
        out[idx] += wt[:, None] * y
    return out.reshape(B, S, H)


def kernel(hidden_states, router_weight, gate_up_proj, gate_up_proj_bias,
           down_proj, down_proj_bias):
    in_maps, idx_lists, wt_lists, C, has_bias = prepare_in_maps(
        hidden_states, router_weight, gate_up_proj, gate_up_proj_bias,
        down_proj, down_proj_bias)
    nc = _get_kernel(C, has_bias)
    last_err = None
    for attempt in range(3):
        try:
            res = run_bass_kernel_spmd(nc, in_maps,
                                       core_ids=list(range(N_CORES)))
            break
        except Exception as e:  # transient device/runtime hiccups
            last_err = e
            if attempt == 2:
                raise
            time.sleep(5 * (attempt + 1))
    return combine(res.results, idx_lists, wt_lists, C, down_proj_bias)
